# revision 24
# baseline (speedup 1.0000x reference)
"""Trainium2 kernel for nn_LinearKalmanFilter.

Math: the reference scan is
    x_t = xp_t @ M_t + (ym_t - bfy) @ Lc_t^T,   xp_t = x_{t-1} @ Wfx + u_t Wfu + d_t Wfd + b
with M_t = I - Wfy @ Lc_t^T and gain Lc_t = L_{t-1} coming from a covariance
recursion that is independent of the data and the batch. Hence
    x_t = x_{t-1} @ A_t + c_t,   A_t = Wfx @ M_t
is an affine-linear recursion with data-independent transition matrices, and
(with x_init = x0 broadcast)
    x_final = x0 @ (A_0 S_0) + sum_t c_t @ S_t,  S_t = A_{t+1}..A_{T-1}.
Substituting c_t gives per-timestep effective input maps
    Gu_t = Wfu M_t S_t, Gd_t = Wfd M_t S_t, Gy_t = Lc_t^T S_t,
    g_t  = (b M_t - bfy Lc_t^T) S_t
so  x_final[b] = sum_t ( u_t[b] Gu_t + d_t[b] Gd_t + ym_t[b] Gy_t ) + sum_t g_t.

The G's are precomputed on host in float64. Because the closed loop is
stable (spectral radius ~0.74 for the shipped weights), ||G_t|| decays
geometrically with T-t and only the last K timesteps contribute above float
precision; K is chosen at runtime from the exact norms (trailing 64-norm
window <= 1e-6 => dropped mass bound ~1e-5 absolute worst case; for the
shipped seed K = 64 with dropped mass ~1.5e-7). The covariance recursion
itself converges (Riccati) in ~60 steps, detected at 1e-15, so host work is
O(K + conv) small matrix products, not O(T).

Device work: one contraction  X^T[j,b] = sum_r Gbig[r,j] * Zbig[r,b]  over
r = (t,k) rows, sharded row-wise across 8 NeuronCores (each core produces a
[128,256] partial, summed on host). G and Z rows are packed side by side in
one "gz" DRAM tensor so each chunk needs a single DMA. Matmuls are plain
fp32 (exact): float32r would be 4x fewer PE cycles but is a rounded format
on real HW (measured rel err 1.5e-4 vs 4.4e-7) and PE time hides under DMA
here anyway.

Raw Bass (no TileContext): this walrus build allows at most ONE embedded
semaphore wait per instruction, which Tile's auto-sync (multi-wait tail
drain, DMA-queue FIFO + slot-release waits) violates. Explicit standalone
wait_ge instructions sidestep the limit; <= 8 total DMAs so each lands on
its own HWDGE queue (no FIFO waits), and all chunk tiles stay resident (no
slot-release waits).
"""

import os
import sys
import numpy as np

for _p in ("/opt/trn_rl_repo", "/root/.axon_site/_ro/trn_rl_repo"):
    if os.path.isdir(_p) and _p not in sys.path:
        sys.path.insert(0, _p)

from concourse import bass, mybir  # noqa: E402
from concourse.bass_utils import run_bass_kernel_spmd  # noqa: E402

N_CORES = 8
# stop the backward pass when the trailing 64-step sum of ||G_t||_F drops
# below this; dropped-contribution bound ~ tol * max_b ||z_tb|| ~ 1e-5 abs.
STOP_WINDOW_TOL = 1e-6
MIN_K = 64
CONV_TOL = 1e-15  # Riccati convergence detection (relative, f64)

# test.py introspection: last BassKernelResults + cost-model estimate.
last_run = None
last_sim_ns = None

# bass programs are shape-keyed and reusable across kernel() calls
_built_cache = {}


def _precompute_G(T, Wfx, bfx, Wfu, bfu, Wfd, bfd, Wfy, bfy, Q, R, P0, L0, x0):
    """Returns (G [K, NZ, NX] f64 for the last K steps, gsum [NX] f64, K).

    K is a multiple of 64 (or T). gsum includes the (batch-independent) bias
    and x0 contributions of the KEPT window; dropped steps are bounded by
    STOP_WINDOW_TOL * ||z|| which is below fp32 resolution of the result.
    """
    f = np.float64
    NX = Wfx.shape[0]
    NY = Wfy.shape[1]
    NU = Wfu.shape[0]
    ND = Wfd.shape[0]
    NZ = NU + ND + NY
    Wfx, Wfu, Wfd, Wfy = (a.astype(f) for a in (Wfx, Wfu, Wfd, Wfy))
    Q, R, P0, L0 = (a.astype(f) for a in (Q, R, P0, L0))
    b = (bfx + bfu + bfd).astype(f)
    bfy = bfy.astype(f)
    eye = np.eye(NX, dtype=f)

    # --- forward covariance recursion; gain used at step t is Lc_t = L_{t-1}.
    # The Riccati iteration converges quickly; after convergence Lc is const.
    Lc_list = [L0]
    P = P0.copy()
    converged = False
    for t in range(T - 1):
        Pp = Wfx @ (P @ Wfx.T) + Q
        PpWfy = Pp @ Wfy
        S = R + Wfy.T @ PpWfy
        L = np.linalg.solve(S.T, PpWfy.T).T
        P = eye - L @ (Wfy.T @ Pp)
        d = np.linalg.norm(L - Lc_list[-1])
        Lc_list.append(L)
        if d <= CONV_TOL * max(np.linalg.norm(L), 1e-300):
            converged = True
            break
    L_inf = Lc_list[-1]

    def Lc(t):
        return Lc_list[t] if t < len(Lc_list) else L_inf

    # --- backward suffix products with early stop once the trailing window
    # of ||G_t||_F is negligible (only valid once we are in the converged
    # regime; without convergence we must walk all the way down to t=0).
    G_rev = []  # G_t for t = T-1, T-2, ...
    norms = []
    gsum = np.zeros(NX, dtype=f)
    S_t = eye.copy()
    MS = None
    t = T - 1
    while t >= 0:
        LcT = Lc(t).T
        Gy = LcT @ S_t
        MS = S_t - Wfy @ Gy  # M_t @ S_t
        Gt = np.empty((NZ, NX), dtype=f)
        Gt[:NU] = Wfu @ MS
        Gt[NU:NU + ND] = Wfd @ MS
        Gt[NU + ND:] = Gy
        G_rev.append(Gt)
        norms.append(np.linalg.norm(Gt))
        gsum += b @ MS - bfy @ Gy
        K = len(G_rev)
        if (
            converged
            and K >= MIN_K
            and K % 64 == 0
            and t > len(Lc_list)  # strictly inside the converged regime
            and sum(norms[-64:]) <= STOP_WINDOW_TOL
        ):
            break
        if t > 0:
            S_t = Wfx @ MS
        t -= 1

    K = len(G_rev)
    if K == T:
        # full window: include the x0 @ A_0 S_0 term (x0 is [1,NX], broadcast
        # over batch -> batch-independent)
        gsum += x0[0].astype(f) @ (Wfx @ MS)
    elif K - 64 >= MIN_K:
        # the early-stop fired because the trailing 64-step window is itself
        # negligible (sum ||G_t|| <= STOP_WINDOW_TOL): don't ship it to the
        # device. Its (negligible) bias contribution stays in gsum.
        G_rev = G_rev[:K - 64]
        K -= 64
    G = np.stack(G_rev[::-1], axis=0)  # [K, NZ, NX], chronological
    return G, gsum, K


def _build_bass(R, B, NX, n_in_dmas=5, n_warmup=3):
    """Per-core program: gz [R, NX+B] rows (g | z) -> out [NX, B] partial.

    n_warmup dummy matmuls (separate PSUM bank, result discarded, no data
    dependency) run during the DMA prefix so the PE p-state/HAM ramp happens
    before the real accumulation chain; fp32 matmuls are 4 cy/row, so cold
    vs warm is ~2x on the 7-matmul chain.
    """
    from contextlib import ExitStack

    nt = R // 128
    assert nt * 128 == R
    assert NX <= 128 and B <= 512  # stationary cols / one PSUM bank (fp32)
    f32 = mybir.dt.float32
    W = NX + B
    nc = bass.Bass()
    gz_ext = nc.declare_dram_parameter("gz", [R, W], f32, isOutput=False)
    out_ext = nc.declare_dram_parameter("out", [NX, B], f32, isOutput=True)

    gz_v = gz_ext.rearrange("(n p) m -> p n m", p=128)

    n_chunks = min(n_in_dmas, nt)
    base, rem = divmod(nt, n_chunks)
    chunks = []
    i = 0
    for ci in range(n_chunks):
        cn = base + (1 if ci < rem else 0)
        chunks.append((i, cn))
        i += cn

    with ExitStack() as ctx:
        gz_sb = ctx.enter_context(nc.sbuf_tensor([128, nt, W], f32))
        out_sb = ctx.enter_context(nc.sbuf_tensor([128, B], f32))
        acc = ctx.enter_context(nc.psum_tensor([128, B], f32))
        junk = ctx.enter_context(nc.psum_tensor([128, B], f32))
        ld_sems = [
            ctx.enter_context(nc.semaphore(f"ld_sem{ci}"))
            for ci in range(n_chunks)
        ]
        pe_sem = ctx.enter_context(nc.semaphore("pe_sem"))
        dve_sem = ctx.enter_context(nc.semaphore("dve_sem"))
        out_sem = ctx.enter_context(nc.semaphore("out_sem"))
        block = ctx.enter_context(nc.Block())

        @block.sync
        def _(sync):
            for ci, (i0, cn) in enumerate(chunks):
                sync.dma_start(
                    out=gz_sb[:, i0:i0 + cn, :], in_=gz_v[:, i0:i0 + cn, :]
                ).then_inc(ld_sems[ci], 16)
            sync.wait_ge(dve_sem, 1)
            sync.dma_start(out=out_ext[:], in_=out_sb[:]).then_inc(out_sem, 16)
            sync.wait_ge(out_sem, 16)

        @block.tensor
        def _(tensor):
            # warmups read out_sb (uninitialized, but NOT concurrently
            # DMA-written like gz_sb would be) into a junk PSUM bank
            for _w in range(n_warmup):
                tensor.matmul(
                    junk[:], out_sb[:, :NX], out_sb[:, :B],
                    start=True, stop=True,
                )
            n = 0
            mm = None
            for ci, (i0, cn) in enumerate(chunks):
                tensor.wait_ge(ld_sems[ci], 16)
                for j in range(cn):
                    mm = tensor.matmul(
                        acc[:],
                        gz_sb[:, i0 + j, :NX],
                        gz_sb[:, i0 + j, NX:],
                        start=(n == 0),
                        stop=(n == nt - 1),
                    )
                    n += 1
            mm.then_inc(pe_sem, 1)

        @block.vector
        def _(vector):
            vector.wait_ge(pe_sem, 1)
            vector.tensor_copy(out_sb[:], acc[:]).then_inc(dve_sem, 1)

    return nc


def kernel(**inputs):
    global last_run, last_sim_ns
    Yp = np.asarray(inputs["Yp"], dtype=np.float32)
    Up = np.asarray(inputs["Up"], dtype=np.float32)
    Dp = np.asarray(inputs["Dp"], dtype=np.float32)
    T, B, NY = Yp.shape
    NU = Up.shape[2]
    ND = Dp.shape[2]
    NX = np.asarray(inputs["Wfx"]).shape[0]
    NZ = NU + ND + NY

    G, gsum, K = _precompute_G(
        T,
        *(np.asarray(inputs[k]) for k in (
            "Wfx", "bfx", "Wfu", "bfu", "Wfd", "bfd", "Wfy", "bfy",
            "Q", "R", "P0", "L0", "x0")),
    )
    t0 = T - K

    if (K * NZ) % (N_CORES * 128) != 0:
        # shapes that don't tile evenly: pad K*NZ rows up with zeros
        Rtot = -(-(K * NZ) // (N_CORES * 128)) * (N_CORES * 128)
    else:
        Rtot = K * NZ

    # packed panel: per row r=(t,k): [ G[t,k,:] | Z[t,k,:] ] with Z rows per t
    # being [u^T; d^T; ym^T] (matches the G row-block order)
    GZ = np.zeros((Rtot, NX + B), dtype=np.float32)
    GZ[:K * NZ, :NX] = G.astype(np.float32).reshape(K * NZ, NX)
    Zp = GZ[:K * NZ, NX:].reshape(K, NZ, B)
    Zp[:, :NU] = Up[t0:].transpose(0, 2, 1)
    Zp[:, NU:NU + ND] = Dp[t0:].transpose(0, 2, 1)
    Zp[:, NU + ND:] = Yp[t0:].transpose(0, 2, 1)

    Rc = Rtot // N_CORES

    # SBUF holds ~112 resident [128, NX+B] f32 tiles; split into passes if over.
    MAX_TILES = 112
    ntc = Rc // 128
    n_pass = (ntc + MAX_TILES - 1) // MAX_TILES
    trace = os.environ.get("KALMAN_TRACE", "0") == "1"
    acc = np.zeros((NX, B), dtype=np.float64)
    built = _built_cache
    done = 0
    for p in range(n_pass):
        pt = min(MAX_TILES, ntc - p * MAX_TILES)
        Rp = pt * 128
        if (Rp, B, NX) not in built:
            built[(Rp, B, NX)] = _build_bass(Rp, B, NX)
        in_maps = []
        for c in range(N_CORES):
            r0 = c * Rc + done
            in_maps.append({"gz": np.ascontiguousarray(GZ[r0:r0 + Rp])})
        res = run_bass_kernel_spmd(built[(Rp, B, NX)], in_maps,
                                   core_ids=list(range(N_CORES)))
        last_run = res
        for c in range(N_CORES):
            acc += res.results[c]["out"].astype(np.float64)
        done += Rp

    if trace:
        try:
            from concourse.timeline_sim import TimelineSim
            used = {
                (min(MAX_TILES, ntc - p * MAX_TILES) * 128, B, NX)
                for p in range(n_pass)
            }
            last_sim_ns = sum(
                TimelineSim(built[k], no_exec=True).simulate() for k in used
            ) * n_pass / len(used)
        except Exception:
            last_sim_ns = None

    x = acc.T + gsum[None, :]
    return x.astype(np.float32)
